# revision 5
# baseline (speedup 1.0000x reference)
"""Additive (Bahdanau) attention kernel for 8 Trainium2 NeuronCores.

Math (per batch b):
    scores[q,k] = sum_d scale[d] * tanh(query[b,q,d] + value[b,k,d])
    out[b,q,:]  = softmax_k(scores) @ value[b]

Sharding: data-parallel over (B=2) x (Tq split 4 ways) -> 8 shards of 256
query rows each; every core holds the full value[b] (256KB) for its batch.

Per-core device program (all fp32):
  - V2  [128,1024] SBUF: value[b].T stacked twice on the partition axis
    (rows 0:64 and 64:128 both hold V^T[d,k]).
  - For each pair j of query rows (q_j, q_{j+128}):
      ACT:  tanh_t = tanh(V2 + bias) where bias[p] = q_j[d] / q_{j+128}[d]
            (per-partition bias column QB[:,j]) -> [128,1024], the
            dominant cost (Tq/2 activations over 128x1024).
      PE:   scores = sblk.T @ tanh_t -> [2,1024] in PSUM, where
            sblk[0:64,0]=scale, sblk[64:128,1]=scale (the sum over d).
      DMA:  row-scatter PSUM [2,1024] -> scores_sb1[j,:], scores_sb2[j,:].
  - Softmax without max-subtraction (|scores| <= sum|scale| ~ 5, exp is
    safe in fp32): W = exp(scores_sb) on ACT.
  - PE-transpose W into W^T chunks [128k,128q]; matmul2 accumulates
    out[q, 0:65] = sum_k W^T.T @ [V | 1] -- the ones column yields the
    softmax denominator for free; normalize with DVE reciprocal.
"""

import os
from contextlib import ExitStack

import numpy as np

import concourse.bass as bass  # noqa: F401  (engine types referenced via nc)
import concourse.mybir as mybir
import concourse.tile as tile
from concourse import bacc
from concourse.bass_utils import run_bass_kernel_spmd

B, TQ, TK, D = 2, 1024, 1024, 64
N_CORES = 8
QCHUNK = (B * TQ) // N_CORES  # 256 query rows per core
PAIRS = QCHUNK // 2  # 128
KCHUNKS = TK // 128  # 8
F32 = mybir.dt.float32
AF = mybir.ActivationFunctionType

# test.py toggles these for profiling
TRACE = False
TRACE_KWARGS: dict = {}
LAST_RESULT = None

_NC = None


def _build_nc():
    nc = bacc.Bacc("TRN2", target_bir_lowering=False, debug=False)

    v2_d = nc.dram_tensor("v2", [128, TK], F32, kind="ExternalInput").ap()
    qb_d = nc.dram_tensor("qb", [128, PAIRS], F32, kind="ExternalInput").ap()
    sblk_d = nc.dram_tensor("sblk", [128, 32], F32, kind="ExternalInput").ap()
    v65_d = nc.dram_tensor("v65", [KCHUNKS, 128, 65], F32, kind="ExternalInput").ap()
    id_d = nc.dram_tensor("ident", [128, 128], F32, kind="ExternalInput").ap()
    out_d = nc.dram_tensor("out", [QCHUNK, D], F32, kind="ExternalOutput").ap()

    with tile.TileContext(nc) as tc, ExitStack() as ctx:
        const = ctx.enter_context(tc.tile_pool(name="const", bufs=1))
        scores = ctx.enter_context(tc.tile_pool(name="scores", bufs=1))
        tanh_pool = ctx.enter_context(tc.tile_pool(name="tanh_pool", bufs=4))
        stage_pool = ctx.enter_context(tc.tile_pool(name="stage_pool", bufs=2))
        w_pool = ctx.enter_context(tc.tile_pool(name="w_pool", bufs=2))
        wt_pool = ctx.enter_context(tc.tile_pool(name="wt_pool", bufs=4))
        small = ctx.enter_context(tc.tile_pool(name="small", bufs=4))
        sc_ps_pool = ctx.enter_context(tc.tile_pool(name="sc_ps", bufs=2, space="PSUM"))
        wt_ps_pool = ctx.enter_context(tc.tile_pool(name="wt_ps", bufs=2, space="PSUM"))
        mm2_ps_pool = ctx.enter_context(
            tc.tile_pool(name="mm2_ps", bufs=1, space="PSUM")
        )

        # ---- load constants -------------------------------------------------
        v2_sb = const.tile([128, TK], F32)
        qb_sb = const.tile([128, PAIRS], F32)
        sblk_sb = const.tile([128, 32], F32)
        ident_sb = const.tile([128, 128], F32)
        v65_sb = const.tile([128, KCHUNKS * 65], F32)
        nc.sync.dma_start(v2_sb[:], v2_d[:])
        nc.sync.dma_start(qb_sb[:], qb_d[:])
        nc.sync.dma_start(sblk_sb[:], sblk_d[:])
        nc.sync.dma_start(ident_sb[:], id_d[:])
        for c in range(KCHUNKS):
            nc.sync.dma_start(v65_sb[:, c * 65 : (c + 1) * 65], v65_d[c])

        sb1 = scores.tile([128, TK], F32)  # scores for q rows 0..127
        sb2 = scores.tile([128, TK], F32)  # scores for q rows 128..255

        # ---- main loop: tanh + scale-contraction per query pair -------------
        # 4 pairs share one PSUM tile at partition offsets 0/32/64/96 (PE
        # column tiling) so eviction to SBUF is one DVE copy per 4 pairs,
        # then two strided row-scatter DMAs distribute rows into sb1/sb2.
        for g in range(PAIRS // 4):
            ps = sc_ps_pool.tile([128, TK], F32, name="ps")
            for i in range(4):
                j = 4 * g + i
                th = tanh_pool.tile([128, TK], F32, name="th")
                nc.scalar.activation(
                    th[:], v2_sb[:], AF.Tanh, bias=qb_sb[:, j : j + 1]
                )
                p0 = 32 * i
                nc.tensor.matmul(
                    ps[p0 : p0 + 32, 0:512],
                    sblk_sb[:],
                    th[:, 0:512],
                    tile_position=(0, p0),
                )
                nc.tensor.matmul(
                    ps[p0 : p0 + 32, 512:1024],
                    sblk_sb[:],
                    th[:, 512:1024],
                    tile_position=(0, p0),
                )
            st = stage_pool.tile([128, TK], F32, name="st")
            nc.vector.tensor_copy(st[:], ps[:])
            for i in range(4):
                p0 = 32 * i
                nc.sync.dma_start(
                    sb1[4 * g + i : 4 * g + i + 1, :], st[p0 : p0 + 1, :]
                )
                nc.sync.dma_start(
                    sb2[4 * g + i : 4 * g + i + 1, :], st[p0 + 1 : p0 + 2, :]
                )

        # ---- per 128-row block: softmax + weights @ [V|1] -------------------
        for blk, sb in enumerate((sb1, sb2)):
            w = w_pool.tile([128, TK], F32, name="w")
            nc.scalar.activation(w[:], sb[:], AF.Exp)
            mm2 = mm2_ps_pool.tile([128, 65], F32, name="mm2")
            for c in range(KCHUNKS):
                wtp = wt_ps_pool.tile([128, 128], F32, name="wtp")
                nc.tensor.transpose(wtp[:], w[:, c * 128 : (c + 1) * 128], ident_sb[:])
                wts = wt_pool.tile([128, 128], F32, name="wts")
                nc.vector.tensor_copy(wts[:], wtp[:])
                nc.tensor.matmul(
                    mm2[:],
                    wts[:],
                    v65_sb[:, c * 65 : (c + 1) * 65],
                    start=(c == 0),
                    stop=(c == KCHUNKS - 1),
                )
            rc = small.tile([128, 1], F32, name="rc")
            nc.vector.reciprocal(rc[:], mm2[:, 64:65])
            ob = small.tile([128, D], F32, name="ob")
            nc.vector.tensor_scalar(
                ob[:], mm2[:, 0:64], rc[:], None, op0=mybir.AluOpType.mult
            )
            nc.sync.dma_start(out_d[blk * 128 : (blk + 1) * 128, :], ob[:])

    nc.compile()
    return nc


def get_nc():
    global _NC
    if _NC is None:
        _NC = _build_nc()
    return _NC


def make_in_maps(query, value, scale):
    query = np.ascontiguousarray(query, np.float32)
    value = np.ascontiguousarray(value, np.float32)
    scale = np.ascontiguousarray(scale, np.float32)
    ident = np.eye(128, dtype=np.float32)
    in_maps = []
    for core in range(N_CORES):
        b, qc = divmod(core, N_CORES // B)
        q0 = qc * QCHUNK
        qch = query[b, q0 : q0 + QCHUNK, :]  # [256, 64]
        vT = value[b].T  # [64, 1024]
        v2 = np.concatenate([vT, vT], axis=0)  # [128, 1024]
        qb = np.concatenate(
            [qch[0:PAIRS].T, qch[PAIRS : 2 * PAIRS].T], axis=0
        )  # [128, 128]
        sblk = np.zeros((128, 32), np.float32)
        sblk[0:D, 0] = scale
        sblk[D : 2 * D, 1] = scale
        v65 = np.concatenate(
            [value[b], np.ones((TK, 1), np.float32)], axis=1
        ).reshape(KCHUNKS, 128, 65)
        in_maps.append(
            {
                "v2": np.ascontiguousarray(v2),
                "qb": np.ascontiguousarray(qb),
                "sblk": sblk,
                "v65": np.ascontiguousarray(v65),
                "ident": ident,
            }
        )
    return in_maps


def kernel(query, value, scale):
    global LAST_RESULT
    nc = get_nc()
    in_maps = make_in_maps(query, value, scale)
    res = run_bass_kernel_spmd(
        nc,
        in_maps,
        core_ids=list(range(N_CORES)),
        trace=TRACE,
        trace_cores=[0] if TRACE else None,
        **TRACE_KWARGS,
    )
    LAST_RESULT = res
    out = np.empty((B, TQ, D), np.float32)
    for core in range(N_CORES):
        b, qc = divmod(core, N_CORES // B)
        q0 = qc * QCHUNK
        out[b, q0 : q0 + QCHUNK, :] = res.results[core]["out"]
    return out


# revision 6
# speedup vs baseline: 1.2172x; 1.2172x over previous
"""Additive (Bahdanau) attention kernel for 8 Trainium2 NeuronCores.

Math (per batch b):
    scores[q,k] = sum_d scale[d] * tanh(query[b,q,d] + value[b,k,d])
    out[b,q,:]  = softmax_k(scores) @ value[b]

Sharding: data-parallel over (B=2) x (Tq split 4 ways) -> 8 shards of 256
query rows each; every core holds the full value[b] (256KB) for its batch.

Per-core device program (all fp32):
  - V2  [128,1024] SBUF: value[b].T stacked twice on the partition axis
    (rows 0:64 and 64:128 both hold V^T[d,k]).
  - For each pair j of query rows (q_j, q_{j+128}):
      ACT:  tanh_t = tanh(V2 + bias) where bias[p] = q_j[d] / q_{j+128}[d]
            (per-partition bias column QB[:,j]) -> [128,1024], the
            dominant cost (Tq/2 activations over 128x1024).
      PE:   scores = sblk.T @ tanh_t -> [2,1024] in PSUM, where
            sblk[0:64,0]=scale, sblk[64:128,1]=scale (the sum over d).
      DMA:  row-scatter PSUM [2,1024] -> scores_sb1[j,:], scores_sb2[j,:].
  - Softmax without max-subtraction (|scores| <= sum|scale| ~ 5, exp is
    safe in fp32): W = exp(scores_sb) on ACT.
  - PE-transpose W into W^T chunks [128k,128q]; matmul2 accumulates
    out[q, 0:65] = sum_k W^T.T @ [V | 1] -- the ones column yields the
    softmax denominator for free; normalize with DVE reciprocal.
"""

import os
from contextlib import ExitStack

import numpy as np

import concourse.bass as bass  # noqa: F401  (engine types referenced via nc)
import concourse.mybir as mybir
import concourse.tile as tile
from concourse import bacc
from concourse.bass_utils import run_bass_kernel_spmd

B, TQ, TK, D = 2, 1024, 1024, 64
N_CORES = 8
QCHUNK = (B * TQ) // N_CORES  # 256 query rows per core
PAIRS = QCHUNK // 2  # 128
KCHUNKS = TK // 128  # 8
F32 = mybir.dt.float32
AF = mybir.ActivationFunctionType

# test.py toggles these for profiling
TRACE = False
TRACE_KWARGS: dict = {}
LAST_RESULT = None

_NC = None


def _build_nc():
    nc = bacc.Bacc("TRN2", target_bir_lowering=False, debug=False)

    v2_d = nc.dram_tensor("v2", [128, TK], F32, kind="ExternalInput").ap()
    qb_d = nc.dram_tensor("qb", [128, PAIRS], F32, kind="ExternalInput").ap()
    sblk_d = nc.dram_tensor("sblk", [128, 32], F32, kind="ExternalInput").ap()
    v65_d = nc.dram_tensor("v65", [KCHUNKS, 128, 65], F32, kind="ExternalInput").ap()
    id_d = nc.dram_tensor("ident", [128, 128], F32, kind="ExternalInput").ap()
    out_d = nc.dram_tensor("out", [QCHUNK, D], F32, kind="ExternalOutput").ap()

    with tile.TileContext(nc) as tc, ExitStack() as ctx:
        const = ctx.enter_context(tc.tile_pool(name="const", bufs=1))
        scores = ctx.enter_context(tc.tile_pool(name="scores", bufs=1))
        tanh_pool = ctx.enter_context(tc.tile_pool(name="tanh_pool", bufs=6))
        stage_pool = ctx.enter_context(tc.tile_pool(name="stage_pool", bufs=2))
        w_pool = ctx.enter_context(tc.tile_pool(name="w_pool", bufs=1))
        wt_pool = ctx.enter_context(tc.tile_pool(name="wt_pool", bufs=4))
        small = ctx.enter_context(tc.tile_pool(name="small", bufs=4))
        sc_ps_pool = ctx.enter_context(tc.tile_pool(name="sc_ps", bufs=2, space="PSUM"))
        wt_ps_pool = ctx.enter_context(tc.tile_pool(name="wt_ps", bufs=2, space="PSUM"))
        mm2_ps_pool = ctx.enter_context(
            tc.tile_pool(name="mm2_ps", bufs=1, space="PSUM")
        )

        # ---- load constants -------------------------------------------------
        v2_sb = const.tile([128, TK], F32)
        qb_sb = const.tile([128, PAIRS], F32)
        sblk_sb = const.tile([128, 32], F32)
        ident_sb = const.tile([128, 128], F32)
        v65_sb = const.tile([128, KCHUNKS * 65], F32)
        nc.sync.dma_start(v2_sb[:], v2_d[:])
        nc.sync.dma_start(qb_sb[:], qb_d[:])
        nc.sync.dma_start(sblk_sb[:], sblk_d[:])
        nc.sync.dma_start(ident_sb[:], id_d[:])
        for c in range(KCHUNKS):
            nc.sync.dma_start(v65_sb[:, c * 65 : (c + 1) * 65], v65_d[c])

        # row j: cols 0:1024 = scores(q_j), cols 1024:2048 = scores(q_{j+128})
        sbB = scores.tile([128, 2 * TK], F32)

        # ---- main loop: tanh + scale-contraction per query pair -------------
        # 4 pairs share one PSUM tile at partition offsets 0/32/64/96 (PE
        # column tiling) so eviction to SBUF is one DVE copy per 4 pairs,
        # then two strided row-scatter DMAs distribute rows into sb1/sb2.
        for g in range(PAIRS // 4):
            ps = sc_ps_pool.tile([128, TK], F32, name="ps")
            for i in range(4):
                j = 4 * g + i
                th = tanh_pool.tile([128, TK], F32, name="th")
                nc.scalar.activation(
                    th[:], v2_sb[:], AF.Tanh, bias=qb_sb[:, j : j + 1]
                )
                p0 = 32 * i
                nc.tensor.matmul(
                    ps[p0 : p0 + 32, 0:512],
                    sblk_sb[:],
                    th[:, 0:512],
                    tile_position=(0, p0),
                )
                nc.tensor.matmul(
                    ps[p0 : p0 + 32, 512:1024],
                    sblk_sb[:],
                    th[:, 512:1024],
                    tile_position=(0, p0),
                )
            st = stage_pool.tile([128, TK], F32, name="st")
            nc.vector.tensor_copy(st[:], ps[:])
            for i in range(4):
                j = 4 * g + i
                p0 = 32 * i
                eng = nc.sync if j % 2 == 0 else nc.gpsimd
                eng.dma_start(sbB[j : j + 1, :], st[p0 : p0 + 2, :])

        # ---- per 128-row block: softmax + weights @ [V|1] -------------------
        w = w_pool.tile([128, 2 * TK], F32, name="w")
        nc.scalar.activation(w[:], sbB[:], AF.Exp)
        for blk in range(2):
            mm2 = mm2_ps_pool.tile([128, 65], F32, name="mm2")
            for c in range(KCHUNKS):
                wtp = wt_ps_pool.tile([128, 128], F32, name="wtp")
                nc.tensor.transpose(
                    wtp[:],
                    w[:, blk * TK + c * 128 : blk * TK + (c + 1) * 128],
                    ident_sb[:],
                )
                wts = wt_pool.tile([128, 128], F32, name="wts")
                nc.vector.tensor_copy(wts[:], wtp[:])
                nc.tensor.matmul(
                    mm2[:],
                    wts[:],
                    v65_sb[:, c * 65 : (c + 1) * 65],
                    start=(c == 0),
                    stop=(c == KCHUNKS - 1),
                )
            rc = small.tile([128, 1], F32, name="rc")
            nc.vector.reciprocal(rc[:], mm2[:, 64:65])
            ob = small.tile([128, D], F32, name="ob")
            nc.vector.tensor_scalar(
                ob[:], mm2[:, 0:64], rc[:], None, op0=mybir.AluOpType.mult
            )
            nc.sync.dma_start(out_d[blk * 128 : (blk + 1) * 128, :], ob[:])

    nc.compile()
    return nc


def get_nc():
    global _NC
    if _NC is None:
        _NC = _build_nc()
    return _NC


def make_in_maps(query, value, scale):
    query = np.ascontiguousarray(query, np.float32)
    value = np.ascontiguousarray(value, np.float32)
    scale = np.ascontiguousarray(scale, np.float32)
    ident = np.eye(128, dtype=np.float32)
    in_maps = []
    for core in range(N_CORES):
        b, qc = divmod(core, N_CORES // B)
        q0 = qc * QCHUNK
        qch = query[b, q0 : q0 + QCHUNK, :]  # [256, 64]
        vT = value[b].T  # [64, 1024]
        v2 = np.concatenate([vT, vT], axis=0)  # [128, 1024]
        qb = np.concatenate(
            [qch[0:PAIRS].T, qch[PAIRS : 2 * PAIRS].T], axis=0
        )  # [128, 128]
        sblk = np.zeros((128, 32), np.float32)
        sblk[0:D, 0] = scale
        sblk[D : 2 * D, 1] = scale
        v65 = np.concatenate(
            [value[b], np.ones((TK, 1), np.float32)], axis=1
        ).reshape(KCHUNKS, 128, 65)
        in_maps.append(
            {
                "v2": np.ascontiguousarray(v2),
                "qb": np.ascontiguousarray(qb),
                "sblk": sblk,
                "v65": np.ascontiguousarray(v65),
                "ident": ident,
            }
        )
    return in_maps


def kernel(query, value, scale):
    global LAST_RESULT
    nc = get_nc()
    in_maps = make_in_maps(query, value, scale)
    res = run_bass_kernel_spmd(
        nc,
        in_maps,
        core_ids=list(range(N_CORES)),
        trace=TRACE,
        trace_cores=[0] if TRACE else None,
        **TRACE_KWARGS,
    )
    LAST_RESULT = res
    out = np.empty((B, TQ, D), np.float32)
    for core in range(N_CORES):
        b, qc = divmod(core, N_CORES // B)
        q0 = qc * QCHUNK
        out[b, q0 : q0 + QCHUNK, :] = res.results[core]["out"]
    return out
